# revision 25
# baseline (speedup 1.0000x reference)
"""F1-loss kernel for Trainium2, data-parallel over 8 NeuronCores.

Strategy (per core, shard of N/8 = 250k rows):
  - SP streams y_pred tiles [128, T*46] fp32 from HBM.
  - GPSIMD replicates labels 46x along the free dim (dense bf16).
  - DVE builds onehot bf16 via is_equal(iota_const, label_rep).
  - ACT casts y_pred fp32 -> bf16 into 48-wide slots with a persistent ones
    column.
  - TensorE accumulates out[46, 47] = onehot^T @ [y_pred_bf16 | 1] in PSUM over
    all 128-row tiles: diag -> tp, col 46 -> counts, host row-sum -> col_sum
    (exact: onehot rows are a partition of unity; padded rows use label -1 so
    their onehot row is all-zero and contributes nothing).
  - Host gathers the 8 [46,47] partials and finishes the O(C) F1 epilogue.

Raw-bass Block style with explicit semaphores: this container's walrus allows
exactly ONE sync-wait per instruction, so all cross-engine waits are standalone
wait_ge instructions (legal), and data instructions carry none.

Engine budget per core (~): DMA 46 MB / ~358 GB/s = 130 us (bound), DVE ~50 us,
ACT ~50-85 us, GPSIMD ~76 us, PE ~90-120 us.
"""

import sys

if "/opt/trn_rl_repo" not in sys.path:
    sys.path.insert(0, "/opt/trn_rl_repo")

from contextlib import ExitStack

import numpy as np

N_CORES = 8
N = 2_000_000
C = 46
P = 128
T = 128                     # 128-row tiles per group
SHARD = N // N_CORES        # 250_000
EPS = 1e-7
NBUF = 3

TRACE = False               # set by test harness to collect HW exec time
LAST_RESULTS = None

_cache = {}


def _build_params(n_rows: int, t: int, mult: int = 1):
    import concourse.bass as bass
    import concourse.mybir as mybir

    rpg = P * t
    g_total = (n_rows + rpg - 1) // rpg

    nc = bass.Bass()
    y_pred = nc.declare_dram_parameter(
        "y_pred", [n_rows, C], mybir.dt.float32, isOutput=False
    )
    # host-permuted labels: yt[p, g*t + b*4 + q] = label of shard row
    # g*rpg + b*512 + 4p + q  (loaded once, 8KB/partition)
    yt = nc.declare_dram_parameter(
        "yt", [P, g_total * t], mybir.dt.float32, isOutput=False
    )
    stats = nc.declare_dram_parameter(
        "stats", [C, C + 1], mybir.dt.float32, isOutput=True
    )

    bf16 = mybir.dt.bfloat16
    f32 = mybir.dt.float32

    # per-group geometry: 512-row blocks of 4 rows per partition (>=512B
    # DMA runs); each block = 4 matmul tiles (46-wide slices of the slot)
    assert t % 4 == 0 and n_rows % 4 == 0
    bpg = t // 4              # blocks per group
    geo = []
    for g in range(g_total):
        rows = min(rpg, n_rows - g * rpg)
        nbf = rows // (4 * P)             # full 512-row blocks
        prem = rows - nbf * 4 * P
        assert prem % 4 == 0
        pp = prem // 4                    # partitions in the partial block
        ntiles = 4 * nbf + (4 if pp else 0)
        geo.append((rows, nbf, pp, ntiles))
    # cumulative per-slot DMA-instruction counts through group g
    yp_dmas = []   # value ACT waits for on s_yp[gg % NBUF], indexed by gg
    slot_yp = [0] * NBUF
    for gg in range(mult * g_total):
        rows, nbf, pp, ntiles = geo[gg % g_total]
        j = gg % NBUF
        slot_yp[j] += (1 if nbf else 0) + (1 if pp else 0)
        yp_dmas.append(slot_yp[j])

    with ExitStack() as ctx:
        e = ctx.enter_context

        iota_f = e(nc.sbuf_tensor("iota_f", [P, t, C], bf16))
        yp_b = [
            e(nc.sbuf_tensor(f"yp{j}", [P, bpg, 4 * C], f32)) for j in range(NBUF)
        ]
        yts_all = e(nc.sbuf_tensor("yts_all", [P, g_total * t], f32))
        rep_b = [e(nc.sbuf_tensor(f"rep{j}", [P, t, C], bf16)) for j in range(NBUF)]
        oh_b = [e(nc.sbuf_tensor(f"oh{j}", [P, t, C], bf16)) for j in range(NBUF)]
        rhs_b = [e(nc.sbuf_tensor(f"rhs{j}", [P, t, C + 2], bf16)) for j in range(NBUF)]
        out_sb = e(nc.sbuf_tensor("out_sb", [C, C + 1], f32))
        ps = e(nc.psum_tensor([C, C + 1], f32))

        s_yp = [e(nc.semaphore(f"s_yp{j}")) for j in range(NBUF)]
        s_yt = e(nc.semaphore("s_yt"))
        s_iota = e(nc.semaphore("s_iota"))
        s_init = e(nc.semaphore("s_init"))
        s_rep = e(nc.semaphore("s_rep"))
        s_oh = e(nc.semaphore("s_oh"))
        s_rhs = e(nc.semaphore("s_rhs"))
        s_mm = e(nc.semaphore("s_mm"))
        s_stat = e(nc.semaphore("s_stat"))

        block = e(nc.Block())

        @block.sync
        def _(sync):
            sync.dma_start(out=yts_all[:, :], in_=yt[:, :]).then_inc(s_yt, 16)
            for gg in range(mult * g_total):
                g = gg % g_total
                rows, nbf, pp, ntiles = geo[g]
                j = gg % NBUF
                if gg >= NBUF:
                    # yp_b[j] free once iteration gg-NBUF's cast is done
                    sync.wait_ge(s_rhs, gg - NBUF + 1)
                row0 = g * rpg
                if nbf:
                    src = y_pred[row0 : row0 + nbf * 4 * P, :].rearrange(
                        "(b p q) c -> p b (q c)", p=P, q=4
                    )
                    sync.dma_start(out=yp_b[j][:, 0:nbf, :], in_=src).then_inc(
                        s_yp[j], 16
                    )
                if pp:
                    src_tail = y_pred[row0 + nbf * 4 * P : row0 + rows, :].rearrange(
                        "(p q) c -> p (q c)", q=4
                    )
                    sync.dma_start(
                        out=yp_b[j][0:pp, nbf, :], in_=src_tail
                    ).then_inc(s_yp[j], 16)
            sync.wait_ge(s_stat, 1)
            sync.dma_start(out=stats[:, :], in_=out_sb[:, :]).then_inc(s_stat, 16)

        @block.gpsimd
        def _(gpsimd):
            gpsimd.iota(
                iota_f[:, :, :],
                pattern=[[0, t], [1, C]],
                channel_multiplier=0,
                allow_small_or_imprecise_dtypes=True,  # 0..45 exact in bf16
            ).then_inc(s_iota, 1)
            gpsimd.wait_ge(s_yt, 16)
            for gg in range(mult * g_total):
                g = gg % g_total
                rows, nbf, pp, ntiles = geo[g]
                j = gg % NBUF
                if gg >= NBUF:
                    gpsimd.wait_ge(s_oh, gg - NBUF + 1)  # rep_j's old reader done
                bc = (
                    yts_all[:, g * t : g * t + ntiles]
                    .unsqueeze(2)
                    .to_broadcast((P, ntiles, C))
                )
                gpsimd.tensor_copy(rep_b[j][:, 0:ntiles, :], bc).then_inc(s_rep, 1)

        @block.vector
        def _(vector):
            for j in range(NBUF):
                ins = vector.memset(rhs_b[j][:, :, C : C + 1], 1.0)
            ins.then_inc(s_init, 1)
            vector.wait_ge(s_iota, 1)
            for gg in range(mult * g_total):
                g = gg % g_total
                rows, nbf, pp, ntiles = geo[g]
                j = gg % NBUF
                vector.wait_ge(s_rep, gg + 1)
                if gg >= NBUF:
                    vector.wait_ge(s_mm, gg - NBUF + 1)  # oh_j's old reader done
                vector.tensor_tensor(
                    oh_b[j][:, 0:ntiles, :],
                    iota_f[:, 0:ntiles, :],
                    rep_b[j][:, 0:ntiles, :],
                    mybir.AluOpType.is_equal,
                ).then_inc(s_oh, 1)
            vector.wait_ge(s_mm, mult * g_total)
            vector.tensor_copy(out_sb[:, :], ps[:, :]).then_inc(s_stat, 1)

        @block.scalar
        def _(scalar):
            for gg in range(mult * g_total):
                g = gg % g_total
                rows, nbf, pp, ntiles = geo[g]
                j = gg % NBUF
                scalar.wait_ge(s_yp[j], 16 * yp_dmas[gg])
                if gg >= NBUF:
                    scalar.wait_ge(s_mm, gg - NBUF + 1)  # rhs_j's old reader done
                last = None
                if nbf:
                    last = scalar.activation(
                        rhs_b[j][:, 0 : 4 * nbf, 0:C],
                        yp_b[j][:, 0:nbf, :].rearrange(
                            "p b (q c) -> p (b q) c", c=C
                        ),
                        mybir.ActivationFunctionType.Copy,
                    )
                if pp:
                    last = scalar.activation(
                        rhs_b[j][0:pp, 4 * nbf : 4 * nbf + 4, 0:C],
                        yp_b[j][0:pp, nbf, :].rearrange("p (q c) -> p q c", c=C),
                        mybir.ActivationFunctionType.Copy,
                    )
                last.then_inc(s_rhs, 1)

        @block.tensor
        def _(tensor):
            tensor.wait_ge(s_init, 1)
            n_iter = mult * g_total
            for gg in range(n_iter):
                g = gg % g_total
                rows, nbf, pp, ntiles = geo[g]
                j = gg % NBUF
                tensor.wait_ge(s_oh, gg + 1)
                tensor.wait_ge(s_rhs, gg + 1)
                for tt in range(ntiles):
                    k = P if tt < 4 * nbf else pp
                    ins = tensor.matmul(
                        ps[:, :],
                        lhsT=oh_b[j][0:k, tt, :],
                        rhs=rhs_b[j][0:k, tt, 0 : C + 1],
                        start=(gg == 0 and tt == 0),
                        stop=(gg == n_iter - 1 and tt == ntiles - 1),
                    )
                ins.then_inc(s_mm, 1)

    return nc


def _prep_labels(y_true_shard: np.ndarray, n_rows: int, t: int) -> np.ndarray:
    rpg = P * t
    g_total = (n_rows + rpg - 1) // rpg
    yt = np.full(g_total * rpg, -1.0, dtype=np.float32)
    yt[:n_rows] = y_true_shard.astype(np.float32)
    # row g*rpg + b*512 + 4p + q  ->  yt[p, g*t + b*4 + q]
    yt = yt.reshape(g_total, t // 4, P, 4).transpose(2, 0, 1, 3)
    return np.ascontiguousarray(yt.reshape(P, g_total * t))


def kernel(y_pred: np.ndarray, y_true: np.ndarray) -> np.ndarray:
    global LAST_RESULTS
    from concourse.bass_utils import run_bass_kernel_spmd

    if "nc" not in _cache:
        _cache["nc"] = _build_params(SHARD, T)
    nc = _cache["nc"]

    y_pred = np.asarray(y_pred)
    y_true = np.asarray(y_true)
    in_maps = []
    for i in range(N_CORES):
        lo = i * SHARD
        in_maps.append(
            {
                "y_pred": np.ascontiguousarray(y_pred[lo : lo + SHARD]),
                "yt": _prep_labels(y_true[lo : lo + SHARD], SHARD, T),
            }
        )

    res = run_bass_kernel_spmd(nc, in_maps, list(range(N_CORES)), trace=TRACE)
    LAST_RESULTS = res

    S = np.zeros((C, C + 1), dtype=np.float64)
    for i in range(N_CORES):
        S += res.results[i]["stats"].astype(np.float64)

    M = S[:, :C]
    counts = S[:, C]
    tp = np.diag(M).copy()
    col_sum = M.sum(axis=0)

    precision = tp / (col_sum + EPS)  # tp + fp = col_sum
    recall = tp / (counts + EPS)      # tp + fn = counts
    f1 = 2.0 * precision * recall / (precision + recall + EPS)
    f1 = np.clip(f1, EPS, 1.0 - EPS)
    return np.asarray(1.0 - f1.mean(), dtype=np.float32)
